# revision 38
# baseline (speedup 1.0000x reference)
"""Trainium2 Bass kernel for CLIPAttention (B=32, S=512, E=768, H=12, D=64).

Strategy: data-parallel over batch across 8 NeuronCores (4 batches/core).
All matmul operands are fp16 (PSUM accumulates fp32); fp16 stationary
operands get fast-weight-load, which fp32/f32r weights do not.

Per batch:
  x -> cast fp16 -> xT (PE transpose) -> qT/kT feature-major + v token-major
  projections. Attention per head, with scores computed TRANSPOSED (k-major)
  so no transpose of the probabilities is ever needed:
    scoresT[k,q] = kh.T @ qh    (PE, triangular: only blocks with k <= q)
    pE = exp(scale * scoresT)   (ACT, written straight to SBUF as fp16)
    diagonal block masked by multiplying with an upper-triangular 0/1 tile
    den[q] = ones.T @ pE        (PE matmuls accumulating over k-tiles)
    po = v_h.T @ pE             (PE, triangular; unnormalized - the per-q
                                 normalization factors out of the k-sum)
    rden = approx-recip(den)    (DVE) -> broadcast to 128 partitions (GPSIMD)
    outT copy = po * rden       (DVE, fused into the PSUM->SBUF copy;
                                 partition-shifted writes put odd heads at
                                 partitions 64:127 directly)
  Final projection back to token-major; biases folded into PSUM->SBUF copies.
"""

import os
import time

import numpy as np
from contextlib import ExitStack

import concourse.bass as bass
import concourse.mybir as mybir
import concourse.tile as tile
from concourse import bacc
from concourse.bass_utils import run_bass_kernel_spmd
from concourse.masks import make_identity, make_upper_triangular

B, S, E, H, D = 32, 512, 768, 12, 64
NCORES = 8
NB = B // NCORES          # batches per core
P = 128
KT = E // P               # 6 feature tiles
QT = S // P               # 4 token tiles
SCALE = float(D) ** -0.5  # 0.125
F32 = mybir.dt.float32
F16 = mybir.dt.float16

AF = mybir.ActivationFunctionType
OP = mybir.AluOpType


def _build():
    nc = bacc.Bacc(trn_type="TRN2")

    hs = nc.dram_tensor("hs", [NB, S, E], F32, kind="ExternalInput")
    w_dr = {}
    b_dr = {}
    for nm in ("q", "k", "v", "o"):
        w_dr[nm] = nc.dram_tensor(f"W{nm}", [E, E], F32, kind="ExternalInput")
        b_dr[nm] = nc.dram_tensor(f"b{nm}", [E], F32, kind="ExternalInput")
    out = nc.dram_tensor("out", [NB, S, E], F32, kind="ExternalOutput")

    with ExitStack() as ctx:
        tc = ctx.enter_context(tile.TileContext(nc))

        singles = ctx.enter_context(tc.tile_pool(name="singles", bufs=1))
        wldpool = ctx.enter_context(tc.tile_pool(name="wldpool", bufs=2))
        xpool = ctx.enter_context(tc.tile_pool(name="xpool", bufs=2))
        x16pool = ctx.enter_context(tc.tile_pool(name="x16pool", bufs=2))
        xtpool = ctx.enter_context(tc.tile_pool(name="xtpool", bufs=2))
        qkvpool = ctx.enter_context(tc.tile_pool(name="qkvpool", bufs=2))
        pepool = ctx.enter_context(tc.tile_pool(name="pepool", bufs=8))
        rpool = ctx.enter_context(tc.tile_pool(name="rpool", bufs=4))
        otpool = ctx.enter_context(tc.tile_pool(name="otpool", bufs=2))
        opool = ctx.enter_context(tc.tile_pool(name="opool", bufs=2))

        ps_mm = ctx.enter_context(tc.tile_pool(name="ps_mm", bufs=2, space="PSUM"))
        ps_s = ctx.enter_context(tc.tile_pool(name="ps_s", bufs=3, space="PSUM"))
        ps_pv = ctx.enter_context(tc.tile_pool(name="ps_pv", bufs=2, space="PSUM"))
        ps_den = ctx.enter_context(tc.tile_pool(name="ps_den", bufs=1, space="PSUM"))

        # ---- constants ----
        ident16 = singles.tile([P, P], F16, name="ident16")
        make_identity(nc, ident16)
        # upper-triangular (incl diagonal) 0/1 mask: keeps q >= k entries of
        # a k-major diagonal block
        triu01 = singles.tile([P, P], F16, name="triu01")
        make_upper_triangular(nc, triu01, val=1.0, diag=True)
        ones16 = singles.tile([P, 1], F16, name="ones16")
        nc.vector.memset(ones16, 1.0)

        # weights in SBUF as fp16 (DMA fp32 -> cast-copy, one-time)
        w_sb = {}
        for wi, nm in enumerate(("q", "k", "v", "o")):
            w_sb[nm] = singles.tile([P, KT, E], F16, name=f"w_{nm}")
            w_view = w_dr[nm].rearrange("(ko p) m -> p ko m", p=P)
            for kk in range(KT):
                wtmp = wldpool.tile([P, E], F32, name=f"wt_{nm}_{kk}", tag="wld")
                nc.sync.dma_start(out=wtmp, in_=w_view[:, kk, :])
                cp = nc.vector.tensor_copy if (wi + kk) % 2 else nc.scalar.copy
                cp(out=w_sb[nm][:, kk, :], in_=wtmp)

        # per-partition bias form for feature-major q/k
        bias_pp = {}
        for nm in ("q", "k"):
            bias_pp[nm] = singles.tile([P, KT], F32, name=f"bpp_{nm}")
            nc.sync.dma_start(
                out=bias_pp[nm], in_=b_dr[nm].rearrange("(ko p) -> p ko", p=P)
            )
        # broadcast-to-all-partitions bias form for token-major v/o
        bias_bc = {}
        for nm in ("v", "o"):
            bias_bc[nm] = singles.tile([P, E], F32, name=f"bbc_{nm}")
            src = b_dr[nm][:]
            bcast = bass.AP(tensor=src.tensor, offset=src.offset, ap=[[0, P], *src.ap])
            nc.sync.dma_start(out=bias_bc[nm], in_=bcast)

        NSPLIT = 384  # N-tile for the two token-major projections (768 = 2x384)
        HN = NSPLIT // D  # heads per N-chunk group = 6

        for b in range(NB):
            # ---- stage A: load x, cast fp16, transpose to feature-major xT ----
            xt = xtpool.tile([P, KT, S], F16, name=f"xt_{b}", tag="xt")
            for i in range(QT):
                x_t = xpool.tile([P, E], F32, name=f"x_{b}_{i}", tag="x")
                nc.sync.dma_start(out=x_t, in_=hs[b, i * P:(i + 1) * P, :])
                x16 = x16pool.tile([P, E], F16, name=f"x16_{b}_{i}", tag="x16")
                nc.scalar.copy(out=x16, in_=x_t)
                for half in range(2):
                    tpx = ps_s.tile([P, S], F16, name=f"tpx_{b}_{i}_{half}", tag="s")
                    for jj in range(3):
                        j = 3 * half + jj
                        nc.tensor.transpose(
                            tpx[:, jj * P:(jj + 1) * P],
                            x16[:, j * P:(j + 1) * P],
                            ident16,
                        )
                    nc.vector.tensor_copy(
                        out=xt[:, 3 * half:3 * half + 3, i * P:(i + 1) * P],
                        in_=tpx[:, :3 * P].rearrange("p (j c) -> p j c", c=P),
                    )

            # ---- stage B: qT, kT feature-major [768, 512] ----
            qkv = {}
            for nm in ("q", "k"):
                dst = qkvpool.tile([P, KT, S], F16, name=f"{nm}T_{b}", tag=f"{nm}T")
                qkv[nm] = dst
                for m in range(KT):
                    ps = ps_mm.tile([P, S], F32, name=f"ps{nm}_{b}_{m}", tag="mm")
                    for kk in range(KT):
                        nc.tensor.matmul(
                            ps,
                            lhsT=w_sb[nm][:, kk, m * P:(m + 1) * P],
                            rhs=xt[:, kk, :],
                            start=(kk == 0),
                            stop=(kk == KT - 1),
                        )
                    if m % 2 == 0:
                        nc.scalar.activation(
                            out=dst[:, m, :],
                            in_=ps,
                            func=AF.Identity,
                            bias=bias_pp[nm][:, m:m + 1],
                            scale=1.0,
                        )
                    else:
                        nc.vector.tensor_scalar_add(
                            out=dst[:, m, :],
                            in0=ps,
                            scalar1=bias_pp[nm][:, m:m + 1],
                        )

            # ---- stage C: v token-major [512, 768] ----
            v_t = qkvpool.tile([P, QT, E], F16, name=f"v_{b}", tag="v")
            for i in range(QT):
                for n in range(E // NSPLIT):
                    ps = ps_mm.tile([P, S], F32, name=f"psv_{b}_{i}_{n}", tag="mm")
                    for kk in range(KT):
                        nc.tensor.matmul(
                            ps[:, :NSPLIT],
                            lhsT=xt[:, kk, i * P:(i + 1) * P],
                            rhs=w_sb["v"][:, kk, n * NSPLIT:(n + 1) * NSPLIT],
                            start=(kk == 0),
                            stop=(kk == KT - 1),
                        )
                    nc.vector.tensor_tensor(
                        out=v_t[:, i, n * NSPLIT:(n + 1) * NSPLIT],
                        in0=ps[:, :NSPLIT],
                        in1=bias_bc["v"][:, n * NSPLIT:(n + 1) * NSPLIT],
                        op=OP.add,
                    )

            # ---- stage D: attention heads (k-major probs, no transposes) ----
            # software-pipelined by one head: head h+1's scores/exp are
            # emitted before head h's PV so the PE always has independent
            # matmul work while the exp chain runs
            outT = otpool.tile([P, KT, S], F16, name=f"outT_{b}", tag="outT")
            pE_live = {}

            def emit_scores(h):
                g, rr = h // 2, h % 2
                qh = qkv["q"][rr * D:(rr + 1) * D, g, :]
                kh = qkv["k"][rr * D:(rr + 1) * D, g, :]
                pE = pepool.tile([P, QT, S], F16, name=f"pE_{b}_{h}", tag="pE")
                pE_live[h] = pE
                for j in range(QT):
                    q0 = j * P
                    n_mm = S - q0
                    ps = ps_s.tile([P, S], F32, name=f"pss_{b}_{h}_{j}", tag="s")
                    nc.tensor.matmul(
                        ps[:, :n_mm],
                        lhsT=kh[:, j * P:(j + 1) * P],
                        rhs=qh[:, q0:],
                        start=True,
                        stop=True,
                    )
                    nc.scalar.activation(
                        out=pE[:, j, q0:],
                        in_=ps[:, :n_mm],
                        func=AF.Exp,
                        scale=SCALE,
                    )
                    # causal mask on the diagonal block: keep q >= k
                    nc.vector.tensor_tensor(
                        out=pE[:, j, q0:q0 + P],
                        in0=pE[:, j, q0:q0 + P],
                        in1=triu01,
                        op=OP.mult,
                    )

            def emit_pv(h):
                g, rr = h // 2, h % 2
                pE = pE_live.pop(h)
                # denominator: ones.T @ pE accumulated over k-tiles
                den = ps_den.tile([1, S], F32, name=f"den_{b}_{h}", tag="den")
                for j in range(QT):
                    nc.tensor.matmul(
                        den[:, j * P:],
                        lhsT=ones16,
                        rhs=pE[:, j, j * P:],
                        start=(j == 0),
                        stop=(j == QT - 1),
                        skip_group_check=True,
                    )
                # PV, unnormalized, triangular over valid k-ranges
                po = ps_pv.tile([D, S], F32, name=f"po_{b}_{h}", tag="pv")
                for j in range(QT):
                    nc.tensor.matmul(
                        po[:, j * P:],
                        lhsT=v_t[:, j, h * D:(h + 1) * D],
                        rhs=pE[:, j, j * P:],
                        start=(j == 0),
                        stop=(j == QT - 1),
                        skip_group_check=True,
                    )
                rden = rpool.tile([1, S], F32, name=f"rden_{b}_{h}", tag="rden")
                nc.vector.reciprocal_approx_fast(rden, den)
                rb = rpool.tile([P, S], F32, name=f"rb_{b}_{h}", tag="rb")
                nc.gpsimd.partition_broadcast(rb, rden)
                # normalization fused into the PSUM->SBUF copy; the write is
                # partition-shifted for odd heads (engines support src/dst
                # partition bases differing)
                nc.vector.tensor_tensor(
                    out=outT[rr * D:(rr + 1) * D, g, :],
                    in0=po,
                    in1=rb[0:D, :],
                    op=OP.mult,
                )

            for h in range(H + 2):
                if h < H:
                    emit_scores(h)
                if h >= 2:
                    emit_pv(h - 2)

            # ---- stage E: final projection, token-major out ----
            for i in range(QT):
                o_t = opool.tile([P, E], F32, name=f"o_{b}_{i}", tag="o")
                for n in range(E // NSPLIT):
                    ps = ps_mm.tile([P, S], F32, name=f"pso_{b}_{i}_{n}", tag="mm")
                    for kk in range(KT):
                        nc.tensor.matmul(
                            ps[:, :NSPLIT],
                            lhsT=outT[:, kk, i * P:(i + 1) * P],
                            rhs=w_sb["o"][:, kk, n * NSPLIT:(n + 1) * NSPLIT],
                            start=(kk == 0),
                            stop=(kk == KT - 1),
                        )
                    nc.vector.tensor_tensor(
                        out=o_t[:, n * NSPLIT:(n + 1) * NSPLIT],
                        in0=ps[:, :NSPLIT],
                        in1=bias_bc["o"][:, n * NSPLIT:(n + 1) * NSPLIT],
                        op=OP.add,
                    )
                nc.sync.dma_start(out=out[b, i * P:(i + 1) * P, :], in_=o_t)

    nc.compile()
    return nc


_NC_CACHE = None


def _get_nc():
    global _NC_CACHE
    if _NC_CACHE is None:
        _NC_CACHE = _build()
    return _NC_CACHE


def run(inputs, trace=False):
    if trace:
        os.environ.pop("BASS_NEVER_TRACE", None)
    else:
        # keep the spmd runner off the NTFF trace path (the profiling hook
        # module is not always present)
        os.environ["BASS_NEVER_TRACE"] = "1"
    hs = np.ascontiguousarray(np.asarray(inputs["hidden_states"], dtype=np.float32))
    assert hs.shape == (B, S, E)
    wb = {}
    for nm in ("q", "k", "v", "o"):
        wb[f"W{nm}"] = np.ascontiguousarray(
            np.asarray(inputs[f"W{nm}"], dtype=np.float32)
        )
        wb[f"b{nm}"] = np.ascontiguousarray(
            np.asarray(inputs[f"b{nm}"], dtype=np.float32)
        )

    nc = _get_nc()
    in_maps = []
    for c in range(NCORES):
        m = {"hs": hs[c * NB:(c + 1) * NB]}
        m.update(wb)
        in_maps.append(m)
    res = run_bass_kernel_spmd(
        nc, in_maps, core_ids=list(range(NCORES)), trace=trace
    )
    outp = np.concatenate([r_["out"] for r_ in res.results], axis=0)
    return outp, res


def kernel(**inputs) -> np.ndarray:
    # retry once on transient accelerator errors (rare NRT exec glitches)
    last = None
    for attempt in range(2):
        try:
            outp, _ = run(inputs, trace=False)
            return outp
        except Exception as e:  # noqa: BLE001
            last = e
            time.sleep(10)
    raise last
